# revision 45
# baseline (speedup 1.0000x reference)
"""Causal multi-head attention (B=2, S=2048, D=1024, H=16) on 8 trn2
NeuronCores.

Sharding (head-parallel): core c handles batch c//4 and heads
4*(c%4) .. 4*(c%4)+3 (a 256-wide slice of the q/k/v feature dim).  W_proj is
tensor-parallel split along the head dim; each core emits a full-shape [S, D]
partial projection output (bf16); the host sums the 4 partials per batch.

Everything on-chip is bf16 (inputs converted on host): bf16 matmuls run at
1 cycle/row at any tile size, DMA bytes halve, and DVE 16-bit fast modes
apply.  Per-core structure:

  - x fed transposed ([d, s]) so the contraction lands on partitions;
    weights load as single DMAs (the SP sequencer serializes DMA issues)
  - q/k for head-pair 0 computed chunk-major so the PE tracks the x DMA
    chunk arrivals; pair-1 q/k and the v projection form a milestone-
    guarded "filler" stream paced evenly across the attention schedule
  - quarters run pair-interleaved ((0,0),(0,1),(1,0),(0,2),(1,1),(0,3),
    (1,2),(1,3)) so the serial ACT exp stream and the per-tile projection
    chains spread across the whole timeline
  - scoresT strips [sk, 2*sq] per head pair with causal raggedness;
    diagonal blocks masked by one bf16 matmul (strict-upper -1000 against
    identity); softmax needs no max subtraction (scores ~ N(0,1))
  - exp on ACT writes et (bf16); the AV matmul consumes et as lhsT
    (contraction = sk) so its output is attn[sq, dh] at only 65 rows per
    accumulation step (64 v cols + 1 ones column for the denominator).
    PSUM start=True lazily zeroes a whole 2KB bank, so only the first
    write into each bank per quarter carries it
  - per-tile normalization (reciprocal + per-partition-scalar muls), PE
    transpose back to [dh, sq], projection + output DMA chained per-tile;
    close-chains are emitted with a lag so the PE FIFO never reaches an
    instruction whose DVE/ACT input is fresh, and the final quarter's
    chains flip to the by-then-idle ACT engine
  - one 8-bank PSUM pool for the whole kernel; window-0 accumulators
    alias the attention-phase tags so there is no pool-transition barrier

The TRN2 ISA holds one sync-wait per instruction; Tile emits more, so
excess waits are hoisted onto same-engine NoOps after scheduling.
"""

import itertools
import os
from collections import deque

import numpy as np

# cache compiled executables (incl. the wrapped NEFF) across processes
os.environ.setdefault("JAX_COMPILATION_CACHE_DIR", "/tmp/jax_comp_cache")
os.environ.setdefault("JAX_PERSISTENT_CACHE_MIN_ENTRY_SIZE_BYTES", "0")
os.environ.setdefault("JAX_PERSISTENT_CACHE_MIN_COMPILE_TIME_SECS", "0")

S = 2048
D = 1024
P = 128
NT = S // P   # 16 sequence tiles
DC = D // P   # 8 contraction chunks
MASK_C = 1000.0
N_CORES = 8
AV_LAG = 3    # steps between emitting scores(j) and AV(j)

_CACHE = {}


def _build_bass():
    import concourse.bass as bass
    import concourse.tile as tile
    from concourse import mybir

    f32 = mybir.dt.float32
    bf16 = mybir.dt.bfloat16
    EXP = mybir.ActivationFunctionType.Exp

    nc = bass.Bass("TRN2")

    xT_d = nc.dram_tensor("xT", [D, S], bf16, kind="ExternalInput")
    wq_d = nc.dram_tensor("wq_t", [D, 256], bf16, kind="ExternalInput")
    wk_d = nc.dram_tensor("wk_t", [D, 256], bf16, kind="ExternalInput")
    wv_d = nc.dram_tensor("wv_t", [D, 256], bf16, kind="ExternalInput")
    wp_d = nc.dram_tensor("wp_t", [256, D], bf16, kind="ExternalInput")
    mask_d = nc.dram_tensor("mask_lhsT", [P, P], bf16, kind="ExternalInput")
    ident_d = nc.dram_tensor("ident", [P, P], bf16, kind="ExternalInput")
    out_d = nc.dram_tensor("out", [S, D], bf16, kind="ExternalOutput")

    with tile.TileContext(nc) as tc:
        with tc.tile_pool(name="persist", bufs=1) as persist:
            xTt = [persist.tile([P, S], bf16, name=f"xTt{c}", tag=f"xTt{c}")
                   for c in range(DC)]
            # weights as single tiles so each loads with ONE dma (the SP
            # sequencer serializes dma issues at 565ns each)
            wq_sb = persist.tile([P, DC, 256], bf16, name="wq_sb", tag="wq_sb")
            wk_sb = persist.tile([P, DC, 256], bf16, name="wk_sb", tag="wk_sb")
            wv_sb = persist.tile([P, DC, 256], bf16, name="wv_sb", tag="wv_sb")
            wp_sb = persist.tile([P, 2, D], bf16, name="wp_sb", tag="wp_sb")
            qT = [persist.tile([P, S], bf16, name=f"qT{p}", tag=f"qT{p}")
                  for p in range(2)]
            kT = [persist.tile([P, S], bf16, name=f"kT{p}", tag=f"kT{p}")
                  for p in range(2)]
            # per sk-tile: 4 heads x [64 v-cols | 1 ones-col]; the ones col
            # makes the AV matmul emit the softmax denominator at col 64
            v4e = [persist.tile([P, 4, 65], bf16, name=f"v4e{t}", tag=f"v4e{t}")
                   for t in range(NT)]
            # normalized attention, [sq, 2 heads x 64] per (pair, sq-tile)
            attn_sb = [[persist.tile([P, P], bf16, name=f"at{p}_{t}",
                                     tag=f"at{p}_{t}") for t in range(NT)]
                       for p in range(2)]
            attnT = [persist.tile([P, S], bf16, name=f"attnT{p}",
                                  tag=f"attnT{p}") for p in range(2)]
            mask_sb = persist.tile([P, P], bf16, name="mask_sb", tag="mask_sb")
            ident_sb = persist.tile([P, P], bf16, name="ident_sb",
                                    tag="ident_sb")

            # DMA order = consumption order; x chunked to pace the
            # chunk-major qk0 loop, weights combined into single transfers.
            wq_r = wq_d.rearrange("(c p) n -> p c n", p=P)
            wk_r = wk_d.rearrange("(c p) n -> p c n", p=P)
            wv_r = wv_d.rearrange("(c p) n -> p c n", p=P)
            wp_r = wp_d.rearrange("(c p) n -> p c n", p=P)
            nc.sync.dma_start(out=wq_sb[:, 0:4, :], in_=wq_r[:, 0:4, :])
            nc.sync.dma_start(out=xTt[0][:, 0:1024], in_=xT_d[0:P, 0:1024])
            nc.sync.dma_start(out=wq_sb[:, 4:8, :], in_=wq_r[:, 4:8, :])
            nc.sync.dma_start(out=xTt[0][:, 1024:S], in_=xT_d[0:P, 1024:S])
            nc.sync.dma_start(out=wk_sb[:], in_=wk_r)
            for c in range(1, DC):
                nc.sync.dma_start(out=xTt[c][:], in_=xT_d[c * P:(c + 1) * P, :])
            nc.sync.dma_start(out=mask_sb[:], in_=mask_d[:])
            nc.sync.dma_start(out=ident_sb[:], in_=ident_d[:])
            nc.sync.dma_start(out=wv_sb[:], in_=wv_r)
            nc.sync.dma_start(out=wp_sb[:], in_=wp_r)

            # One psum pool for the whole kernel (8 banks exactly):
            #   strip: 2 x [P,2,512] f32 (4 banks)  pa: 1 x [P,2,512] (2)
            #   aux1:  1 bank   aux2: 1 bank
            # Window 0 (pair-0 q/k, chunk-major) aliases its 8 psum
            # accumulators onto these same tags so there is no pool
            # transition barrier: the first window-A allocations just WAR on
            # the matching window-0 copies.
            with tc.tile_pool(name="asb", bufs=1) as asb, \
                 tc.tile_pool(name="ps", bufs=1, space="PSUM") as ps:

                qp01 = ps.tile([P, 2, 512], f32, name="qp01", tag="strip",
                               bufs=2)
                qp23 = ps.tile([P, 2, 512], f32, name="qp23", tag="strip",
                               bufs=2)
                kp23 = ps.tile([P, 2, 512], f32, name="kp23", tag="pa")
                kp0 = ps.tile([P, 512], f32, name="kp0", tag="aux1")
                kp1 = ps.tile([P, 512], f32, name="kp1", tag="aux2")
                qp = [qp01[:, 0, :], qp01[:, 1, :], qp23[:, 0, :],
                      qp23[:, 1, :]]
                kp = [kp0[:], kp1[:], kp23[:, 0, :], kp23[:, 1, :]]
                for c in range(DC - 1):
                    for n in range(4):
                        nc.tensor.matmul(
                            qp[n], lhsT=wq_sb[:, c, 0:P],
                            rhs=xTt[c][:, n * 512:(n + 1) * 512],
                            start=(c == 0), stop=False,
                            skip_group_check=True)
                    for n in range(4):
                        nc.tensor.matmul(
                            kp[n], lhsT=wk_sb[:, c, 0:P],
                            rhs=xTt[c][:, n * 512:(n + 1) * 512],
                            start=(c == 0), stop=False,
                            skip_group_check=True)
                # last chunk: finish k/q per n and copy out immediately
                # (k on DVE, q on ACT) so the copies overlap the next n's
                # matmuls and the psum slots free up front-to-back
                c = DC - 1
                for n in range(4):
                    nc.tensor.matmul(
                        kp[n], lhsT=wk_sb[:, c, 0:P],
                        rhs=xTt[c][:, n * 512:(n + 1) * 512],
                        start=False, stop=True, skip_group_check=True)
                    nc.tensor.matmul(
                        qp[n], lhsT=wq_sb[:, c, 0:P],
                        rhs=xTt[c][:, n * 512:(n + 1) * 512],
                        start=False, stop=True, skip_group_check=True)
                    nc.vector.tensor_copy(kT[0][:, n * 512:(n + 1) * 512],
                                          kp[n])
                    nc.scalar.copy(qT[0][:, n * 512:(n + 1) * 512], qp[n])

                def v_tiles(t0, t1):
                    for t in range(t0, t1):
                        vp = ps.tile([P, 256], f32, name="vp", tag="aux1")
                        for c in range(DC):
                            nc.tensor.matmul(
                                vp[:], lhsT=xTt[c][:, t * P:(t + 1) * P],
                                rhs=wv_sb[:, c, :],
                                start=(c == 0), stop=(c == DC - 1),
                                skip_group_check=True)
                            if c < DC - 1:
                                yield 256, None
                        nc.vector.tensor_copy(
                            v4e[t][:, :, 0:64],
                            vp.rearrange("p (h d) -> p h d", h=4))
                        nc.vector.memset(v4e[t][:, :, 64:65], 1.0)
                        yield 256, f"v{t}"

                def qk1_block(key, n):
                    dst, wsb = (kT, wk_sb) if key == "k" else (qT, wq_sb)
                    pp = ps.tile([P, 512], f32, name="pp", tag="aux2")
                    for c in range(DC):
                        nc.tensor.matmul(
                            pp[:], lhsT=wsb[:, c, P:2 * P],
                            rhs=xTt[c][:, n * 512:(n + 1) * 512],
                            start=(c == 0), stop=(c == DC - 1),
                            skip_group_check=True)
                        if c < DC - 1:
                            yield 512, None
                    nc.vector.tensor_copy(
                        dst[1][:, n * 512:(n + 1) * 512], pp[:])
                    yield 512, f"{key}1n{n}"

                def filler_stream():
                    # ordered to match the quarter schedule's consumption;
                    # need() force-drains on any shortfall
                    for g in range(4):
                        yield from v_tiles(4 * g, 4 * g + 4)
                        yield from qk1_block("k", g)
                        yield from qk1_block("q", g)

                filler = filler_stream()
                filler_done = [False]
                fill_ms = set()
                fill_left = [2 * 32768]   # total filler rows (v + qk1)
                steps_left = [104]        # in-loop + trailing steps

                def emit_filler(rows_target):
                    got = 0
                    while got < rows_target and not filler_done[0]:
                        try:
                            r, m = next(filler)
                            got += r
                            if m:
                                fill_ms.add(m)
                        except StopIteration:
                            filler_done[0] = True
                    fill_left[0] -= got
                    return got

                def emit_filler_paced():
                    # spread the remaining filler evenly over the remaining
                    # steps so late quarters keep PE-busy work too
                    steps_left[0] = max(steps_left[0] - 1, 1)
                    emit_filler(fill_left[0] // steps_left[0])

                def need(m):
                    # force-drain filler until milestone m has been emitted
                    # (emission-order dependency: the dependent instruction
                    # must come AFTER the work it reads, or it reads garbage)
                    while m not in fill_ms and not filler_done[0]:
                        emit_filler(1)

                # close-chain stages, deferred so PE never waits on a
                # just-issued DVE/ACT result: stage1 = norm (per-quarter,
                # reads pa), stage2 = transpose + attnT copy, stage3 =
                # proj + out.  chain2/3 carry only tile indices, so they
                # survive across quarters.
                chain2 = deque()
                chain3 = deque()

                def stage2(t, tail):
                    for p in range(2):
                        tp = ps.tile([P, P], bf16, name="tp", tag="aux1")
                        nc.tensor.transpose(
                            tp[:], attn_sb[p][t][:], ident_sb[:])
                        if tail and p == 0:
                            nc.scalar.copy(
                                attnT[p][:, t * P:(t + 1) * P], tp[:])
                        else:
                            nc.vector.tensor_copy(
                                attnT[p][:, t * P:(t + 1) * P], tp[:])
                    chain3.append((t, 0, tail))
                    chain3.append((t, 1, tail))

                def stage3(t, oc, tail):
                    pj = ps.tile([P, 512], f32, name="pj", tag="aux2")
                    for p in range(2):
                        nc.tensor.matmul(
                            pj[:],
                            lhsT=attnT[p][:, t * P:(t + 1) * P],
                            rhs=wp_sb[:, p, oc * 512:(oc + 1) * 512],
                            start=(p == 0), stop=(p == 1),
                            skip_group_check=True)
                    ob = asb.tile([P, 512], bf16, name="ob", tag="ob",
                                  bufs=6)
                    if oc == 1 and tail:
                        nc.scalar.copy(ob[:], pj[:])
                    else:
                        nc.vector.tensor_copy(ob[:], pj[:])
                    nc.sync.dma_start(
                        out=out_d[t * P:(t + 1) * P,
                                  oc * 512:(oc + 1) * 512],
                        in_=ob[:])

                def emit_quarter(pr, qc, do_proj, lag=AV_LAG, tail=False):
                    c0 = qc * 512
                    jmax = min(4 * qc + 3, NT - 1)
                    pa = ps.tile([P, 2, 512], f32, name="pa", tag="pa")
                    ets = {}
                    pend = deque()
                    chain1 = deque()

                    def stage1(t, tt):
                        rc = asb.tile([P, 2, 1], f32, name="rc", tag="rc",
                                      bufs=8)
                        nc.vector.reciprocal(
                            rc[:], pa[:, :, tt * 65 + 64:tt * 65 + 65])
                        for h in range(2):
                            if tail:
                                nc.scalar.mul(
                                    attn_sb[pr][t][:, h * 64:(h + 1) * 64],
                                    pa[:, h, tt * 65:tt * 65 + 64],
                                    rc[:, h, :])
                            else:
                                nc.vector.tensor_scalar_mul(
                                    attn_sb[pr][t][:, h * 64:(h + 1) * 64],
                                    pa[:, h, tt * 65:tt * 65 + 64],
                                    rc[:, h, :])
                        if do_proj:
                            chain2.append((t, tail))

                    def run_chains():
                        if chain3:
                            stage3(*chain3.popleft())
                        if chain2:
                            stage2(*chain2.popleft())
                        while chain1:
                            stage1(*chain1.popleft())

                    def emit_av(j):
                        need(f"v{j}")
                        et = ets.pop(j)
                        for t in range(max(j, 4 * qc), 4 * qc + 4):
                            tt = t - 4 * qc
                            for h in range(2):
                                # start=True lazily zeroes the WHOLE psum
                                # bank (2KB zero region), so only the first
                                # write into each h-bank per quarter may
                                # carry it; later windows zero-fill on
                                # first touch.
                                nc.tensor.matmul(
                                    pa[:, h, tt * 65:tt * 65 + 65],
                                    lhsT=et[:, h, t * P - c0:t * P - c0 + P],
                                    rhs=v4e[j][:, 2 * pr + h, :],
                                    start=(j == 0 and tt == 0),
                                    stop=(j == t),
                                    skip_group_check=True)
                            if j == t:
                                chain1.append((t, tt))

                    for j in range(jmax + 1):
                        if pr == 1:
                            need(f"k1n{j // 4}")
                            need(f"q1n{qc}")
                        w0 = j * P
                        lo = max(w0, c0)
                        w = c0 + 512 - lo
                        strip = ps.tile([P, 2, 512], f32, name="strip",
                                        tag="strip", bufs=2)
                        diag = j // 4 == qc
                        for h in range(2):
                            nc.tensor.matmul(
                                strip[:, h, lo - c0:lo - c0 + w],
                                lhsT=kT[pr][h * 64:(h + 1) * 64, w0:w0 + P],
                                rhs=qT[pr][h * 64:(h + 1) * 64, lo:lo + w],
                                start=True, stop=not diag,
                                skip_group_check=True)
                        if diag:
                            for h in range(2):
                                nc.tensor.matmul(
                                    strip[:, h, w0 - c0:w0 - c0 + P],
                                    lhsT=mask_sb[:], rhs=ident_sb[:],
                                    start=False, stop=True,
                                    skip_group_check=True)
                        et = asb.tile([P, 2, 512], bf16, name="et", tag="et",
                                      bufs=8)
                        nc.scalar.activation(
                            out=et[:, :, lo - c0:lo - c0 + w],
                            in_=strip[:, :, lo - c0:lo - c0 + w],
                            func=EXP)
                        ets[j] = et
                        pend.append(j)
                        if len(pend) > lag:
                            emit_av(pend.popleft())
                        run_chains()
                        emit_filler_paced()
                    while pend:
                        emit_av(pend.popleft())
                        run_chains()
                        emit_filler_paced()
                    return run_chains

                # pair-interleaved quarter order: spreads pair-1 exp (ACT)
                # and the per-tile projection chains across the whole
                # timeline instead of back-loading them
                drain = None
                order = [(0, 0), (0, 1), (1, 0), (0, 2), (1, 1), (0, 3),
                         (1, 2), (1, 3)]
                for pr, qc in order:
                    drain = emit_quarter(pr, qc, do_proj=(pr == 1),
                                         tail=((pr, qc) == order[-1]))
                # drain all remaining close-chain work, stage-batched so
                # each engine gets runs of independent work
                while chain2:
                    stage2(*chain2.popleft())
                while chain3:
                    stage3(*chain3.popleft())
                emit_filler(1 << 30)

    return nc


def _fix_matmul_waits(nc):
    """The TRN2 ISA events struct holds exactly ONE sync-wait per
    instruction and walrus codegen refuses instructions carrying more
    ("Too many sync wait commands").  Tile emits multi-wait instructions,
    so legalize: hoist excess waits onto single-wait NoOps inserted right
    before the instruction on the same engine -- engine FIFO order
    preserves the synchronization semantics."""
    import bass_rust
    import concourse.mybir as mybir

    n = 0
    for bb in nc.main_func.blocks:
        insts = bb.instructions
        i = 0
        while i < len(insts):
            ins = insts[i]
            si = getattr(ins, "sync_info", None)
            if si is not None and len(si.on_wait) >= 2:
                for w in si.on_wait[:-1]:
                    nop = mybir.InstNoOp(name=f"I-xwait-{n}", ins=[], outs=[])
                    nop.engine = ins.engine
                    nop.sync_info = bass_rust.SyncInfo(
                        on_wait=[w], on_update=[])
                    insts.insert(i, nop)
                    n += 1
                    i += 1
                ins.sync_info = bass_rust.SyncInfo(
                    on_wait=[si.on_wait[-1]], on_update=si.on_update)
            i += 1
    return n


def get_nc(legalize=True):
    key = ("nc", legalize)
    if key not in _CACHE:
        nc = _build_bass()
        if legalize:
            _fix_matmul_waits(nc)
        _CACHE[key] = nc
    return _CACHE[key]


def make_in_maps(x, W_q, W_k, W_v, W_proj):
    import ml_dtypes

    bf = ml_dtypes.bfloat16
    x = np.asarray(x, np.float32)
    W_q = np.asarray(W_q, np.float32)
    W_k = np.asarray(W_k, np.float32)
    W_v = np.asarray(W_v, np.float32)
    W_proj = np.asarray(W_proj, np.float32)

    mask = np.triu(np.full((P, P), -MASK_C, np.float32), k=1).astype(bf)
    ident = np.eye(P, dtype=bf)

    xTs = [np.ascontiguousarray(x[b].T).astype(bf) for b in range(2)]
    in_maps = []
    for core in range(N_CORES):
        b = core // 4
        g = core % 4
        rs = slice(g * 256, (g + 1) * 256)
        in_maps.append({
            "xT": xTs[b],
            "wq_t": np.ascontiguousarray(W_q[rs].T / 8.0).astype(bf),
            "wk_t": np.ascontiguousarray(W_k[rs].T).astype(bf),
            "wv_t": np.ascontiguousarray(W_v[rs].T).astype(bf),
            "wp_t": np.ascontiguousarray(W_proj[:, rs].T).astype(bf),
            "mask_lhsT": mask,
            "ident": ident,
        })
    return in_maps


def kernel(x, W_q, W_k, W_v, W_proj, _results_hook=None):
    from concourse.bass_utils import run_bass_kernel_spmd

    nc = get_nc()
    in_maps = make_in_maps(x, W_q, W_k, W_v, W_proj)
    res = run_bass_kernel_spmd(nc, in_maps, core_ids=list(range(N_CORES)))
    if _results_hook is not None:
        _results_hook(res)
    out = np.zeros((2, S, D), np.float32)
    for core in range(N_CORES):
        out[core // 4] += res.results[core]["out"].astype(np.float32)
    return out


if __name__ == "__main__":
    nc = get_nc()
    print("built ok; instructions:",
          sum(len(bb.instructions) for bb in nc.main_func.blocks))


# revision 55
# speedup vs baseline: 1.0057x; 1.0057x over previous
"""Causal multi-head attention (B=2, S=2048, D=1024, H=16) on 8 trn2
NeuronCores.

Sharding (head-parallel): core c handles batch c//4 and heads
4*(c%4) .. 4*(c%4)+3 (a 256-wide slice of the q/k/v feature dim).  W_proj is
tensor-parallel split along the head dim; each core emits a full-shape [S, D]
partial projection output (bf16); the host sums the 4 partials per batch.

Everything on-chip is bf16 (inputs converted on host): bf16 matmuls run at
1 cycle/row at any tile size, DMA bytes halve, and DVE 16-bit fast modes
apply.  Per-core structure:

  - x fed transposed ([d, s]) so the contraction lands on partitions;
    weights load as single DMAs (the SP sequencer serializes DMA issues)
  - q/k for head-pair 0 computed chunk-major so the PE tracks the x DMA
    chunk arrivals; pair-1 q/k and the v projection form a milestone-
    guarded "filler" stream paced evenly across the attention schedule
  - quarters run pair-interleaved ((0,0),(0,1),(1,0),(0,2),(1,1),(0,3),
    (1,2),(1,3)) so the serial ACT exp stream and the per-tile projection
    chains spread across the whole timeline
  - scoresT strips [sk, 2*sq] per head pair with causal raggedness;
    diagonal blocks masked by one bf16 matmul (strict-upper -1000 against
    identity); softmax needs no max subtraction (scores ~ N(0,1))
  - exp on ACT writes et (bf16); the AV matmul consumes et as lhsT
    (contraction = sk) so its output is attn[sq, dh] at only 65 rows per
    accumulation step (64 v cols + 1 ones column for the denominator).
    PSUM start=True lazily zeroes a whole 2KB bank, so only the first
    write into each bank per quarter carries it
  - per-tile normalization (reciprocal + per-partition-scalar muls), PE
    transpose back to [dh, sq], projection + output DMA chained per-tile;
    close-chains are emitted with a lag so the PE FIFO never reaches an
    instruction whose DVE/ACT input is fresh, and the final quarter's
    chains flip to the by-then-idle ACT engine
  - one 8-bank PSUM pool for the whole kernel; window-0 accumulators
    alias the attention-phase tags so there is no pool-transition barrier

The TRN2 ISA holds one sync-wait per instruction; Tile emits more, so
excess waits are hoisted onto same-engine NoOps after scheduling.
"""

import itertools
import os
from collections import deque

import numpy as np

# cache compiled executables (incl. the wrapped NEFF) across processes
os.environ.setdefault("JAX_COMPILATION_CACHE_DIR", "/tmp/jax_comp_cache")
os.environ.setdefault("JAX_PERSISTENT_CACHE_MIN_ENTRY_SIZE_BYTES", "0")
os.environ.setdefault("JAX_PERSISTENT_CACHE_MIN_COMPILE_TIME_SECS", "0")

S = 2048
D = 1024
P = 128
NT = S // P   # 16 sequence tiles
DC = D // P   # 8 contraction chunks
MASK_C = 1000.0
N_CORES = 8
AV_LAG = 3    # steps between emitting scores(j) and AV(j)

_CACHE = {}


def _build_bass():
    import concourse.bass as bass
    import concourse.tile as tile
    from concourse import mybir

    f32 = mybir.dt.float32
    bf16 = mybir.dt.bfloat16
    EXP = mybir.ActivationFunctionType.Exp

    nc = bass.Bass("TRN2")

    xT_d = nc.dram_tensor("xT", [D, S], bf16, kind="ExternalInput")
    wq_d = nc.dram_tensor("wq_t", [D, 256], bf16, kind="ExternalInput")
    wk_d = nc.dram_tensor("wk_t", [D, 256], bf16, kind="ExternalInput")
    wv_d = nc.dram_tensor("wv_t", [D, 256], bf16, kind="ExternalInput")
    wp_d = nc.dram_tensor("wp_t", [256, D], bf16, kind="ExternalInput")
    mask_d = nc.dram_tensor("mask_lhsT", [P, P], bf16, kind="ExternalInput")
    ident_d = nc.dram_tensor("ident", [P, P], bf16, kind="ExternalInput")
    out_d = nc.dram_tensor("out", [S, D], bf16, kind="ExternalOutput")

    with tile.TileContext(nc) as tc:
        with tc.tile_pool(name="persist", bufs=1) as persist:
            xTt = [persist.tile([P, S], bf16, name=f"xTt{c}", tag=f"xTt{c}")
                   for c in range(DC)]
            # weights as single tiles so each loads with ONE dma (the SP
            # sequencer serializes dma issues at 565ns each)
            wq_sb = persist.tile([P, DC, 256], bf16, name="wq_sb", tag="wq_sb")
            wk_sb = persist.tile([P, DC, 256], bf16, name="wk_sb", tag="wk_sb")
            wv_sb = persist.tile([P, DC, 256], bf16, name="wv_sb", tag="wv_sb")
            wp_sb = persist.tile([P, 2, D], bf16, name="wp_sb", tag="wp_sb")
            qT = [persist.tile([P, S], bf16, name=f"qT{p}", tag=f"qT{p}")
                  for p in range(2)]
            kT = [persist.tile([P, S], bf16, name=f"kT{p}", tag=f"kT{p}")
                  for p in range(2)]
            # per sk-tile: 4 heads x [64 v-cols | 1 ones-col]; the ones col
            # makes the AV matmul emit the softmax denominator at col 64
            v4e = [persist.tile([P, 4, 65], bf16, name=f"v4e{t}", tag=f"v4e{t}")
                   for t in range(NT)]
            # normalized attention, [sq, 2 heads x 64] per (pair, sq-tile)
            attn_sb = [[persist.tile([P, P], bf16, name=f"at{p}_{t}",
                                     tag=f"at{p}_{t}") for t in range(NT)]
                       for p in range(2)]
            attnT = [persist.tile([P, S], bf16, name=f"attnT{p}",
                                  tag=f"attnT{p}") for p in range(2)]
            mask_sb = persist.tile([P, P], bf16, name="mask_sb", tag="mask_sb")
            ident_sb = persist.tile([P, P], bf16, name="ident_sb",
                                    tag="ident_sb")

            # DMA order = consumption order; x chunked to pace the
            # chunk-major qk0 loop, weights combined into single transfers.
            wq_r = wq_d.rearrange("(c p) n -> p c n", p=P)
            wk_r = wk_d.rearrange("(c p) n -> p c n", p=P)
            wv_r = wv_d.rearrange("(c p) n -> p c n", p=P)
            wp_r = wp_d.rearrange("(c p) n -> p c n", p=P)
            nc.sync.dma_start(out=wq_sb[:, 0:4, :], in_=wq_r[:, 0:4, :])
            nc.sync.dma_start(out=xTt[0][:, 0:1024], in_=xT_d[0:P, 0:1024])
            nc.sync.dma_start(out=wq_sb[:, 4:8, :], in_=wq_r[:, 4:8, :])
            nc.sync.dma_start(out=xTt[0][:, 1024:S], in_=xT_d[0:P, 1024:S])
            nc.sync.dma_start(out=wk_sb[:], in_=wk_r)
            for c in range(1, DC):
                nc.sync.dma_start(out=xTt[c][:], in_=xT_d[c * P:(c + 1) * P, :])
            nc.sync.dma_start(out=mask_sb[:], in_=mask_d[:])
            nc.sync.dma_start(out=ident_sb[:], in_=ident_d[:])
            nc.sync.dma_start(out=wv_sb[:], in_=wv_r)
            nc.sync.dma_start(out=wp_sb[:], in_=wp_r)

            # One psum pool for the whole kernel (8 banks exactly):
            #   strip: 2 x [P,2,512] f32 (4 banks)  pa: 1 x [P,2,512] (2)
            #   aux1:  1 bank   aux2: 1 bank
            # Window 0 (pair-0 q/k, chunk-major) aliases its 8 psum
            # accumulators onto these same tags so there is no pool
            # transition barrier: the first window-A allocations just WAR on
            # the matching window-0 copies.
            with tc.tile_pool(name="asb", bufs=1) as asb, \
                 tc.tile_pool(name="ps", bufs=1, space="PSUM") as ps:

                qp01 = ps.tile([P, 2, 512], f32, name="qp01", tag="strip",
                               bufs=2)
                qp23 = ps.tile([P, 2, 512], f32, name="qp23", tag="strip",
                               bufs=2)
                kp23 = ps.tile([P, 2, 512], f32, name="kp23", tag="pa")
                kp0 = ps.tile([P, 512], f32, name="kp0", tag="aux1")
                kp1 = ps.tile([P, 512], f32, name="kp1", tag="aux2")
                qp = [qp01[:, 0, :], qp01[:, 1, :], qp23[:, 0, :],
                      qp23[:, 1, :]]
                kp = [kp0[:], kp1[:], kp23[:, 0, :], kp23[:, 1, :]]
                for c in range(DC - 1):
                    for n in range(4):
                        nc.tensor.matmul(
                            qp[n], lhsT=wq_sb[:, c, 0:P],
                            rhs=xTt[c][:, n * 512:(n + 1) * 512],
                            start=(c == 0), stop=False,
                            skip_group_check=True)
                    for n in range(4):
                        nc.tensor.matmul(
                            kp[n], lhsT=wk_sb[:, c, 0:P],
                            rhs=xTt[c][:, n * 512:(n + 1) * 512],
                            start=(c == 0), stop=False,
                            skip_group_check=True)
                # last chunk: finish k/q per n and copy out immediately
                # (k on DVE, q on ACT) so the copies overlap the next n's
                # matmuls and the psum slots free up front-to-back
                c = DC - 1
                for n in range(4):
                    nc.tensor.matmul(
                        kp[n], lhsT=wk_sb[:, c, 0:P],
                        rhs=xTt[c][:, n * 512:(n + 1) * 512],
                        start=False, stop=True, skip_group_check=True)
                    nc.tensor.matmul(
                        qp[n], lhsT=wq_sb[:, c, 0:P],
                        rhs=xTt[c][:, n * 512:(n + 1) * 512],
                        start=False, stop=True, skip_group_check=True)
                    nc.scalar.copy(kT[0][:, n * 512:(n + 1) * 512], kp[n])
                    nc.vector.tensor_copy(qT[0][:, n * 512:(n + 1) * 512],
                                          qp[n])

                def v_tiles(t0, t1):
                    for t in range(t0, t1):
                        vp = ps.tile([P, 256], f32, name="vp", tag="aux1")
                        for c in range(DC):
                            nc.tensor.matmul(
                                vp[:], lhsT=xTt[c][:, t * P:(t + 1) * P],
                                rhs=wv_sb[:, c, :],
                                start=(c == 0), stop=(c == DC - 1),
                                skip_group_check=True)
                            if c < DC - 1:
                                yield 256, None
                        nc.vector.tensor_copy(
                            v4e[t][:, :, 0:64],
                            vp.rearrange("p (h d) -> p h d", h=4))
                        nc.vector.memset(v4e[t][:, :, 64:65], 1.0)
                        yield 256, f"v{t}"

                def qk1_block(key, n):
                    dst, wsb = (kT, wk_sb) if key == "k" else (qT, wq_sb)
                    pp = ps.tile([P, 512], f32, name="pp", tag="aux2")
                    for c in range(DC):
                        nc.tensor.matmul(
                            pp[:], lhsT=wsb[:, c, P:2 * P],
                            rhs=xTt[c][:, n * 512:(n + 1) * 512],
                            start=(c == 0), stop=(c == DC - 1),
                            skip_group_check=True)
                        if c < DC - 1:
                            yield 512, None
                    nc.vector.tensor_copy(
                        dst[1][:, n * 512:(n + 1) * 512], pp[:])
                    yield 512, f"{key}1n{n}"

                def filler_stream():
                    # ordered to match the quarter schedule's consumption;
                    # need() force-drains on any shortfall
                    for g in range(4):
                        yield from v_tiles(4 * g, 4 * g + 4)
                        yield from qk1_block("k", g)
                        yield from qk1_block("q", g)

                filler = filler_stream()
                filler_done = [False]
                fill_ms = set()
                fill_left = [2 * 32768]   # total filler rows (v + qk1)
                steps_left = [104]        # in-loop + trailing steps

                def emit_filler(rows_target):
                    got = 0
                    while got < rows_target and not filler_done[0]:
                        try:
                            r, m = next(filler)
                            got += r
                            if m:
                                fill_ms.add(m)
                        except StopIteration:
                            filler_done[0] = True
                    fill_left[0] -= got
                    return got

                def emit_filler_paced():
                    # spread the remaining filler evenly over the remaining
                    # steps so late quarters keep PE-busy work too
                    steps_left[0] = max(steps_left[0] - 1, 1)
                    emit_filler(fill_left[0] // steps_left[0])

                def need(m):
                    # force-drain filler until milestone m has been emitted
                    # (emission-order dependency: the dependent instruction
                    # must come AFTER the work it reads, or it reads garbage)
                    while m not in fill_ms and not filler_done[0]:
                        emit_filler(1)

                # close-chain stages, deferred so PE never waits on a
                # just-issued DVE/ACT result: stage1 = norm (per-quarter,
                # reads pa), stage2 = transpose + attnT copy, stage3 =
                # proj + out.  chain2/3 carry only tile indices, so they
                # survive across quarters.
                chain2 = deque()
                chain3 = deque()

                def stage2(t, tail):
                    for p in range(2):
                        tp = ps.tile([P, P], bf16, name="tp", tag="aux1")
                        nc.tensor.transpose(
                            tp[:], attn_sb[p][t][:], ident_sb[:])
                        if tail:
                            nc.scalar.copy(
                                attnT[p][:, t * P:(t + 1) * P], tp[:])
                        else:
                            nc.vector.tensor_copy(
                                attnT[p][:, t * P:(t + 1) * P], tp[:])
                    chain3.append((t, 0, tail))
                    chain3.append((t, 1, tail))

                def stage3(t, oc, tail):
                    pj = ps.tile([P, 512], f32, name="pj", tag="aux2")
                    for p in range(2):
                        nc.tensor.matmul(
                            pj[:],
                            lhsT=attnT[p][:, t * P:(t + 1) * P],
                            rhs=wp_sb[:, p, oc * 512:(oc + 1) * 512],
                            start=(p == 0), stop=(p == 1),
                            skip_group_check=True)
                    ob = asb.tile([P, 512], bf16, name="ob", tag="ob",
                                  bufs=6)
                    if oc == 1 and tail:
                        nc.scalar.copy(ob[:], pj[:])
                    else:
                        nc.vector.tensor_copy(ob[:], pj[:])
                    nc.sync.dma_start(
                        out=out_d[t * P:(t + 1) * P,
                                  oc * 512:(oc + 1) * 512],
                        in_=ob[:])

                def emit_quarter(pr, qc, do_proj, lag=AV_LAG, tail=False):
                    c0 = qc * 512
                    jmax = min(4 * qc + 3, NT - 1)
                    pa = ps.tile([P, 2, 512], f32, name="pa", tag="pa")
                    ets = {}
                    pend = deque()
                    chain1 = deque()

                    def stage1(t, tt):
                        rc = asb.tile([P, 2, 1], f32, name="rc", tag="rc",
                                      bufs=8)
                        nc.vector.reciprocal(
                            rc[:], pa[:, :, tt * 65 + 64:tt * 65 + 65])
                        for h in range(2):
                            if tail:
                                nc.scalar.mul(
                                    attn_sb[pr][t][:, h * 64:(h + 1) * 64],
                                    pa[:, h, tt * 65:tt * 65 + 64],
                                    rc[:, h, :])
                            else:
                                nc.vector.tensor_scalar_mul(
                                    attn_sb[pr][t][:, h * 64:(h + 1) * 64],
                                    pa[:, h, tt * 65:tt * 65 + 64],
                                    rc[:, h, :])
                        if do_proj:
                            chain2.append((t, tail))

                    def run_chains():
                        if chain3:
                            stage3(*chain3.popleft())
                        if chain2:
                            stage2(*chain2.popleft())
                        while chain1:
                            stage1(*chain1.popleft())

                    def emit_av(j):
                        need(f"v{j}")
                        et = ets.pop(j)
                        for t in range(max(j, 4 * qc), 4 * qc + 4):
                            tt = t - 4 * qc
                            for h in range(2):
                                # start=True lazily zeroes the WHOLE psum
                                # bank (2KB zero region), so only the first
                                # write into each h-bank per quarter may
                                # carry it; later windows zero-fill on
                                # first touch.
                                nc.tensor.matmul(
                                    pa[:, h, tt * 65:tt * 65 + 65],
                                    lhsT=et[:, h, t * P - c0:t * P - c0 + P],
                                    rhs=v4e[j][:, 2 * pr + h, :],
                                    start=(j == 0 and tt == 0),
                                    stop=(j == t),
                                    skip_group_check=True)
                            if j == t:
                                chain1.append((t, tt))

                    for j in range(jmax + 1):
                        if pr == 1:
                            need(f"k1n{j // 4}")
                            need(f"q1n{qc}")
                        w0 = j * P
                        lo = max(w0, c0)
                        w = c0 + 512 - lo
                        strip = ps.tile([P, 2, 512], f32, name="strip",
                                        tag="strip", bufs=2)
                        diag = j // 4 == qc
                        for h in range(2):
                            nc.tensor.matmul(
                                strip[:, h, lo - c0:lo - c0 + w],
                                lhsT=kT[pr][h * 64:(h + 1) * 64, w0:w0 + P],
                                rhs=qT[pr][h * 64:(h + 1) * 64, lo:lo + w],
                                start=True, stop=not diag,
                                skip_group_check=True)
                        if diag:
                            for h in range(2):
                                nc.tensor.matmul(
                                    strip[:, h, w0 - c0:w0 - c0 + P],
                                    lhsT=mask_sb[:], rhs=ident_sb[:],
                                    start=False, stop=True,
                                    skip_group_check=True)
                        et = asb.tile([P, 2, 512], bf16, name="et", tag="et",
                                      bufs=6)
                        nc.scalar.activation(
                            out=et[:, :, lo - c0:lo - c0 + w],
                            in_=strip[:, :, lo - c0:lo - c0 + w],
                            func=EXP)
                        ets[j] = et
                        pend.append(j)
                        if len(pend) > lag:
                            emit_av(pend.popleft())
                        run_chains()
                        emit_filler_paced()
                    while pend:
                        emit_av(pend.popleft())
                        run_chains()
                        emit_filler_paced()
                    return run_chains

                # pair-interleaved quarter order: spreads pair-1 exp (ACT)
                # and the per-tile projection chains across the whole
                # timeline instead of back-loading them
                drain = None
                order = [(0, 0), (0, 1), (1, 0), (0, 2), (1, 1), (0, 3),
                         (1, 2), (1, 3)]
                for pr, qc in order:
                    drain = emit_quarter(pr, qc, do_proj=(pr == 1),
                                         tail=((pr, qc) == order[-1]))
                # drain all remaining close-chain work, stage-batched so
                # each engine gets runs of independent work
                while chain2:
                    stage2(*chain2.popleft())
                while chain3:
                    stage3(*chain3.popleft())
                emit_filler(1 << 30)

    return nc


def _fix_matmul_waits(nc):
    """The TRN2 ISA events struct holds exactly ONE sync-wait per
    instruction and walrus codegen refuses instructions carrying more
    ("Too many sync wait commands").  Tile emits multi-wait instructions,
    so legalize: hoist excess waits onto single-wait NoOps inserted right
    before the instruction on the same engine -- engine FIFO order
    preserves the synchronization semantics."""
    import bass_rust
    import concourse.mybir as mybir

    n = 0
    for bb in nc.main_func.blocks:
        insts = bb.instructions
        i = 0
        while i < len(insts):
            ins = insts[i]
            si = getattr(ins, "sync_info", None)
            if si is not None and len(si.on_wait) >= 2:
                for w in si.on_wait[:-1]:
                    nop = mybir.InstNoOp(name=f"I-xwait-{n}", ins=[], outs=[])
                    nop.engine = ins.engine
                    nop.sync_info = bass_rust.SyncInfo(
                        on_wait=[w], on_update=[])
                    insts.insert(i, nop)
                    n += 1
                    i += 1
                ins.sync_info = bass_rust.SyncInfo(
                    on_wait=[si.on_wait[-1]], on_update=si.on_update)
            i += 1
    return n


def get_nc(legalize=True):
    key = ("nc", legalize)
    if key not in _CACHE:
        nc = _build_bass()
        if legalize:
            _fix_matmul_waits(nc)
        _CACHE[key] = nc
    return _CACHE[key]


def make_in_maps(x, W_q, W_k, W_v, W_proj):
    import ml_dtypes

    bf = ml_dtypes.bfloat16
    x = np.asarray(x, np.float32)
    W_q = np.asarray(W_q, np.float32)
    W_k = np.asarray(W_k, np.float32)
    W_v = np.asarray(W_v, np.float32)
    W_proj = np.asarray(W_proj, np.float32)

    mask = np.triu(np.full((P, P), -MASK_C, np.float32), k=1).astype(bf)
    ident = np.eye(P, dtype=bf)

    xTs = [np.ascontiguousarray(x[b].T).astype(bf) for b in range(2)]
    in_maps = []
    for core in range(N_CORES):
        b = core // 4
        g = core % 4
        rs = slice(g * 256, (g + 1) * 256)
        in_maps.append({
            "xT": xTs[b],
            "wq_t": np.ascontiguousarray(W_q[rs].T / 8.0).astype(bf),
            "wk_t": np.ascontiguousarray(W_k[rs].T).astype(bf),
            "wv_t": np.ascontiguousarray(W_v[rs].T).astype(bf),
            "wp_t": np.ascontiguousarray(W_proj[:, rs].T).astype(bf),
            "mask_lhsT": mask,
            "ident": ident,
        })
    return in_maps


def kernel(x, W_q, W_k, W_v, W_proj, _results_hook=None):
    from concourse.bass_utils import run_bass_kernel_spmd

    nc = get_nc()
    in_maps = make_in_maps(x, W_q, W_k, W_v, W_proj)
    res = run_bass_kernel_spmd(nc, in_maps, core_ids=list(range(N_CORES)))
    if _results_hook is not None:
        _results_hook(res)
    out = np.zeros((2, S, D), np.float32)
    for core in range(N_CORES):
        out[core // 4] += res.results[core]["out"].astype(np.float32)
    return out


if __name__ == "__main__":
    nc = get_nc()
    print("built ok; instructions:",
          sum(len(bb.instructions) for bb in nc.main_func.blocks))


# revision 67
# speedup vs baseline: 1.0261x; 1.0203x over previous
"""Causal multi-head attention (B=2, S=2048, D=1024, H=16) on 8 trn2
NeuronCores.

Sharding (head-parallel): core c handles batch c//4 and heads
4*(c%4) .. 4*(c%4)+3 (a 256-wide slice of the q/k/v feature dim).  W_proj is
tensor-parallel split along the head dim; each core emits a full-shape [S, D]
partial projection output (bf16); the host sums the 4 partials per batch.

Everything on-chip is bf16 (inputs converted on host): bf16 matmuls run at
1 cycle/row at any tile size, DMA bytes halve, and DVE 16-bit fast modes
apply.  Per-core structure:

  - x fed transposed ([d, s]) so the contraction lands on partitions;
    weights load as single DMAs (the SP sequencer serializes DMA issues)
  - q/k for head-pair 0 computed chunk-major so the PE tracks the x DMA
    chunk arrivals; pair-1 q/k and the v projection form a milestone-
    guarded "filler" stream paced evenly across the attention schedule
  - quarters run pair-interleaved ((0,0),(0,1),(1,0),(0,2),(1,1),(0,3),
    (1,2),(1,3)) so the serial ACT exp stream and the per-tile projection
    chains spread across the whole timeline
  - scoresT strips [sk, 2*sq] per head pair with causal raggedness;
    diagonal blocks masked by one bf16 matmul (strict-upper -1000 against
    identity); softmax needs no max subtraction (scores ~ N(0,1))
  - exp on ACT writes et (bf16); the AV matmul consumes et as lhsT
    (contraction = sk) so its output is attn[sq, dh] at only 65 rows per
    accumulation step (64 v cols + 1 ones column for the denominator).
    PSUM start=True lazily zeroes a whole 2KB bank, so only the first
    write into each bank per quarter carries it
  - per-tile normalization (reciprocal + per-partition-scalar muls), PE
    transpose back to [dh, sq], projection + output DMA chained per-tile;
    close-chains are emitted with a lag so the PE FIFO never reaches an
    instruction whose DVE/ACT input is fresh, and the final quarter's
    chains flip to the by-then-idle ACT engine
  - one 8-bank PSUM pool for the whole kernel; window-0 accumulators
    alias the attention-phase tags so there is no pool-transition barrier

The TRN2 ISA holds one sync-wait per instruction; Tile emits more, so
excess waits are hoisted onto same-engine NoOps after scheduling.
"""

import itertools
import os
from collections import deque

import numpy as np

# cache compiled executables (incl. the wrapped NEFF) across processes
os.environ.setdefault("JAX_COMPILATION_CACHE_DIR", "/tmp/jax_comp_cache")
os.environ.setdefault("JAX_PERSISTENT_CACHE_MIN_ENTRY_SIZE_BYTES", "0")
os.environ.setdefault("JAX_PERSISTENT_CACHE_MIN_COMPILE_TIME_SECS", "0")

S = 2048
D = 1024
P = 128
NT = S // P   # 16 sequence tiles
DC = D // P   # 8 contraction chunks
MASK_C = 1000.0
N_CORES = 8
AV_LAG = 3    # steps between emitting scores(j) and AV(j)

_CACHE = {}


def _build_bass():
    import concourse.bass as bass
    import concourse.tile as tile
    from concourse import mybir

    f32 = mybir.dt.float32
    bf16 = mybir.dt.bfloat16
    EXP = mybir.ActivationFunctionType.Exp

    nc = bass.Bass("TRN2")

    xT_d = nc.dram_tensor("xT", [D, S], bf16, kind="ExternalInput")
    wq_d = nc.dram_tensor("wq_t", [D, 256], bf16, kind="ExternalInput")
    wk_d = nc.dram_tensor("wk_t", [D, 256], bf16, kind="ExternalInput")
    wv_d = nc.dram_tensor("wv_t", [D, 256], bf16, kind="ExternalInput")
    wp_d = nc.dram_tensor("wp_t", [256, D], bf16, kind="ExternalInput")
    mask_d = nc.dram_tensor("mask_lhsT", [P, P], bf16, kind="ExternalInput")
    ident_d = nc.dram_tensor("ident", [P, P], bf16, kind="ExternalInput")
    out_d = nc.dram_tensor("out", [S, D], bf16, kind="ExternalOutput")

    with tile.TileContext(nc) as tc:
        with tc.tile_pool(name="persist", bufs=1) as persist:
            xTt = [persist.tile([P, S], bf16, name=f"xTt{c}", tag=f"xTt{c}")
                   for c in range(DC)]
            # weights as single tiles so each loads with ONE dma (the SP
            # sequencer serializes dma issues at 565ns each)
            wq_sb = persist.tile([P, DC, 256], bf16, name="wq_sb", tag="wq_sb")
            wk_sb = persist.tile([P, DC, 256], bf16, name="wk_sb", tag="wk_sb")
            wv_sb = persist.tile([P, DC, 256], bf16, name="wv_sb", tag="wv_sb")
            wp_sb = persist.tile([P, 2, D], bf16, name="wp_sb", tag="wp_sb")
            qT = [persist.tile([P, S], bf16, name=f"qT{p}", tag=f"qT{p}")
                  for p in range(2)]
            kT = [persist.tile([P, S], bf16, name=f"kT{p}", tag=f"kT{p}")
                  for p in range(2)]
            # per sk-tile: 4 heads x [64 v-cols | 1 ones-col]; the ones col
            # makes the AV matmul emit the softmax denominator at col 64
            v4e = [persist.tile([P, 4, 65], bf16, name=f"v4e{t}", tag=f"v4e{t}")
                   for t in range(NT)]
            # normalized attention, [sq, 2 heads x 64] per (pair, sq-tile)
            attn_sb = [[persist.tile([P, P], bf16, name=f"at{p}_{t}",
                                     tag=f"at{p}_{t}") for t in range(NT)]
                       for p in range(2)]
            attnT = [persist.tile([P, S], bf16, name=f"attnT{p}",
                                  tag=f"attnT{p}") for p in range(2)]
            mask_sb = persist.tile([P, P], bf16, name="mask_sb", tag="mask_sb")
            ident_sb = persist.tile([P, P], bf16, name="ident_sb",
                                    tag="ident_sb")

            # DMA order = consumption order; x chunked to pace the
            # chunk-major qk0 loop, weights combined into single transfers.
            wq_r = wq_d.rearrange("(c p) n -> p c n", p=P)
            wk_r = wk_d.rearrange("(c p) n -> p c n", p=P)
            wv_r = wv_d.rearrange("(c p) n -> p c n", p=P)
            wp_r = wp_d.rearrange("(c p) n -> p c n", p=P)
            nc.sync.dma_start(out=wq_sb[:, 0:4, :], in_=wq_r[:, 0:4, :])
            nc.sync.dma_start(out=xTt[0][:, 0:1024], in_=xT_d[0:P, 0:1024])
            nc.sync.dma_start(out=wq_sb[:, 4:8, :], in_=wq_r[:, 4:8, :])
            nc.sync.dma_start(out=xTt[0][:, 1024:S], in_=xT_d[0:P, 1024:S])
            nc.sync.dma_start(out=wk_sb[:], in_=wk_r)
            for c in range(1, DC):
                nc.sync.dma_start(out=xTt[c][:], in_=xT_d[c * P:(c + 1) * P, :])
            nc.sync.dma_start(out=mask_sb[:], in_=mask_d[:])
            nc.sync.dma_start(out=ident_sb[:], in_=ident_d[:])
            nc.sync.dma_start(out=wv_sb[:], in_=wv_r)
            nc.sync.dma_start(out=wp_sb[:], in_=wp_r)

            # One psum pool for the whole kernel (8 banks exactly):
            #   strip: 2 x [P,2,512] f32 (4 banks)  pa: 1 x [P,2,512] (2)
            #   aux1:  1 bank   aux2: 1 bank
            # Window 0 (pair-0 q/k, chunk-major) aliases its 8 psum
            # accumulators onto these same tags so there is no pool
            # transition barrier: the first window-A allocations just WAR on
            # the matching window-0 copies.
            with tc.tile_pool(name="asb", bufs=1) as asb, \
                 tc.tile_pool(name="ps", bufs=1, space="PSUM") as ps:

                qp01 = ps.tile([P, 2, 512], f32, name="qp01", tag="strip",
                               bufs=2)
                qp23 = ps.tile([P, 2, 512], f32, name="qp23", tag="strip",
                               bufs=2)
                kp23 = ps.tile([P, 2, 512], f32, name="kp23", tag="pa")
                kp0 = ps.tile([P, 512], f32, name="kp0", tag="aux1")
                kp1 = ps.tile([P, 512], f32, name="kp1", tag="aux2")
                qp = [qp01[:, 0, :], qp01[:, 1, :], qp23[:, 0, :],
                      qp23[:, 1, :]]
                kp = [kp0[:], kp1[:], kp23[:, 0, :], kp23[:, 1, :]]
                for c in range(DC - 1):
                    for n in range(4):
                        nc.tensor.matmul(
                            qp[n], lhsT=wq_sb[:, c, 0:P],
                            rhs=xTt[c][:, n * 512:(n + 1) * 512],
                            start=(c == 0), stop=False,
                            skip_group_check=True)
                    for n in range(4):
                        nc.tensor.matmul(
                            kp[n], lhsT=wk_sb[:, c, 0:P],
                            rhs=xTt[c][:, n * 512:(n + 1) * 512],
                            start=(c == 0), stop=False,
                            skip_group_check=True)
                # last chunk: finish k/q per n and copy out immediately
                # (k on DVE, q on ACT) so the copies overlap the next n's
                # matmuls and the psum slots free up front-to-back
                c = DC - 1
                for n in range(4):
                    nc.tensor.matmul(
                        kp[n], lhsT=wk_sb[:, c, 0:P],
                        rhs=xTt[c][:, n * 512:(n + 1) * 512],
                        start=False, stop=True, skip_group_check=True)
                    nc.tensor.matmul(
                        qp[n], lhsT=wq_sb[:, c, 0:P],
                        rhs=xTt[c][:, n * 512:(n + 1) * 512],
                        start=False, stop=True, skip_group_check=True)
                    nc.scalar.copy(kT[0][:, n * 512:(n + 1) * 512], kp[n])
                    nc.vector.tensor_copy(qT[0][:, n * 512:(n + 1) * 512],
                                          qp[n])

                def v_tiles(t0, t1):
                    for t in range(t0, t1):
                        vp = ps.tile([P, 256], f32, name="vp", tag="aux1")
                        for c in range(DC):
                            nc.tensor.matmul(
                                vp[:], lhsT=xTt[c][:, t * P:(t + 1) * P],
                                rhs=wv_sb[:, c, :],
                                start=(c == 0), stop=(c == DC - 1),
                                skip_group_check=True)
                            if c < DC - 1:
                                yield 256, None
                        nc.vector.tensor_copy(
                            v4e[t][:, :, 0:64],
                            vp.rearrange("p (h d) -> p h d", h=4))
                        nc.vector.memset(v4e[t][:, :, 64:65], 1.0)
                        yield 256, f"v{t}"

                def qk1_block(key, n):
                    dst, wsb = (kT, wk_sb) if key == "k" else (qT, wq_sb)
                    pp = ps.tile([P, 512], f32, name="pp", tag="aux2")
                    for c in range(DC):
                        nc.tensor.matmul(
                            pp[:], lhsT=wsb[:, c, P:2 * P],
                            rhs=xTt[c][:, n * 512:(n + 1) * 512],
                            start=(c == 0), stop=(c == DC - 1),
                            skip_group_check=True)
                        if c < DC - 1:
                            yield 512, None
                    nc.vector.tensor_copy(
                        dst[1][:, n * 512:(n + 1) * 512], pp[:])
                    yield 512, f"{key}1n{n}"

                def filler_stream():
                    # ordered to match the quarter schedule's consumption;
                    # need() force-drains on any shortfall
                    for g in range(4):
                        yield from v_tiles(4 * g, 4 * g + 4)
                        yield from qk1_block("k", g)
                        yield from qk1_block("q", g)

                filler = filler_stream()
                filler_done = [False]
                fill_ms = set()
                fill_left = [2 * 32768]   # total filler rows (v + qk1)
                steps_left = [104]        # in-loop + trailing steps

                def emit_filler(rows_target):
                    got = 0
                    while got < rows_target and not filler_done[0]:
                        try:
                            r, m = next(filler)
                            got += r
                            if m:
                                fill_ms.add(m)
                        except StopIteration:
                            filler_done[0] = True
                    fill_left[0] -= got
                    return got

                def emit_filler_paced():
                    # spread the remaining filler evenly over the remaining
                    # steps so late quarters keep PE-busy work too
                    steps_left[0] = max(steps_left[0] - 1, 1)
                    emit_filler(fill_left[0] // steps_left[0])

                def need(m):
                    # force-drain filler until milestone m has been emitted
                    # (emission-order dependency: the dependent instruction
                    # must come AFTER the work it reads, or it reads garbage)
                    while m not in fill_ms and not filler_done[0]:
                        emit_filler(1)

                # close-chain stages, deferred so PE never waits on a
                # just-issued DVE/ACT result: stage1 = norm (per-quarter,
                # reads pa), stage2 = transpose + attnT copy, stage3 =
                # proj + out.  chain2/3 carry only tile indices, so they
                # survive across quarters.
                chain2 = deque()
                chain3 = deque()

                def stage2(t, tail):
                    for p in range(2):
                        tp = ps.tile([P, P], bf16, name="tp", tag="aux1")
                        nc.tensor.transpose(
                            tp[:], attn_sb[p][t][:], ident_sb[:])
                        if tail and p == 0:
                            nc.scalar.copy(
                                attnT[p][:, t * P:(t + 1) * P], tp[:])
                        else:
                            nc.vector.tensor_copy(
                                attnT[p][:, t * P:(t + 1) * P], tp[:])
                    chain3.append((t, 0, tail))
                    chain3.append((t, 1, tail))

                def stage3(t, oc, tail):
                    pj = ps.tile([P, 512], f32, name="pj", tag="aux2")
                    for p in range(2):
                        nc.tensor.matmul(
                            pj[:],
                            lhsT=attnT[p][:, t * P:(t + 1) * P],
                            rhs=wp_sb[:, p, oc * 512:(oc + 1) * 512],
                            start=(p == 0), stop=(p == 1),
                            skip_group_check=True)
                    ob = asb.tile([P, 512], bf16, name="ob", tag="ob",
                                  bufs=6)
                    if oc == 1 and tail:
                        nc.scalar.copy(ob[:], pj[:])
                    else:
                        nc.vector.tensor_copy(ob[:], pj[:])
                    nc.sync.dma_start(
                        out=out_d[t * P:(t + 1) * P,
                                  oc * 512:(oc + 1) * 512],
                        in_=ob[:])

                def emit_quarter(pr, qc, do_proj, lag=AV_LAG, tail=False):
                    c0 = qc * 512
                    jmax = min(4 * qc + 3, NT - 1)
                    pa = ps.tile([P, 2, 512], f32, name="pa", tag="pa")
                    ets = {}
                    pend = deque()
                    chain1 = deque()

                    def stage1(t, tt):
                        rc = asb.tile([P, 2, 1], f32, name="rc", tag="rc",
                                      bufs=8)
                        nc.vector.reciprocal(
                            rc[:], pa[:, :, tt * 65 + 64:tt * 65 + 65])
                        # one fused multiply for both heads via a
                        # zero-stride broadcast of the reciprocals
                        nc.vector.tensor_mul(
                            attn_sb[pr][t].rearrange("p (h d) -> p h d", h=2),
                            pa[:, :, :].rearrange(
                                "p h w -> p h w")[:, :, tt * 65:tt * 65 + 64],
                            rc[:].broadcast_to([P, 2, 64]))
                        if do_proj:
                            chain2.append((t, tail))

                    def run_chains():
                        if chain3:
                            stage3(*chain3.popleft())
                        if chain2:
                            stage2(*chain2.popleft())
                        while chain1:
                            stage1(*chain1.popleft())

                    def emit_av(j):
                        need(f"v{j}")
                        et = ets.pop(j)
                        for t in range(max(j, 4 * qc), 4 * qc + 4):
                            tt = t - 4 * qc
                            for h in range(2):
                                # start=True lazily zeroes the WHOLE psum
                                # bank (2KB zero region), so only the first
                                # write into each h-bank per quarter may
                                # carry it; later windows zero-fill on
                                # first touch.
                                nc.tensor.matmul(
                                    pa[:, h, tt * 65:tt * 65 + 65],
                                    lhsT=et[:, h, t * P - c0:t * P - c0 + P],
                                    rhs=v4e[j][:, 2 * pr + h, :],
                                    start=(j == 0 and tt == 0),
                                    stop=(j == t),
                                    skip_group_check=True)
                            if j == t:
                                chain1.append((t, tt))

                    for j in range(jmax + 1):
                        if pr == 1:
                            need(f"k1n{j // 4}")
                            need(f"q1n{qc}")
                        w0 = j * P
                        lo = max(w0, c0)
                        w = c0 + 512 - lo
                        strip = ps.tile([P, 2, 512], f32, name="strip",
                                        tag="strip", bufs=2)
                        diag = j // 4 == qc
                        for h in range(2):
                            nc.tensor.matmul(
                                strip[:, h, lo - c0:lo - c0 + w],
                                lhsT=kT[pr][h * 64:(h + 1) * 64, w0:w0 + P],
                                rhs=qT[pr][h * 64:(h + 1) * 64, lo:lo + w],
                                start=True, stop=not diag,
                                skip_group_check=True)
                        if diag:
                            for h in range(2):
                                nc.tensor.matmul(
                                    strip[:, h, w0 - c0:w0 - c0 + P],
                                    lhsT=mask_sb[:], rhs=ident_sb[:],
                                    start=False, stop=True,
                                    skip_group_check=True)
                        et = asb.tile([P, 2, 512], bf16, name="et", tag="et",
                                      bufs=8)
                        nc.scalar.activation(
                            out=et[:, :, lo - c0:lo - c0 + w],
                            in_=strip[:, :, lo - c0:lo - c0 + w],
                            func=EXP)
                        ets[j] = et
                        pend.append(j)
                        if len(pend) > lag:
                            emit_av(pend.popleft())
                        run_chains()
                        emit_filler_paced()
                    while pend:
                        emit_av(pend.popleft())
                        run_chains()
                        emit_filler_paced()
                    return run_chains

                # pair-interleaved quarter order: spreads pair-1 exp (ACT)
                # and the per-tile projection chains across the whole
                # timeline instead of back-loading them
                drain = None
                order = [(0, 0), (0, 1), (1, 0), (0, 2), (1, 1), (0, 3),
                         (1, 2), (1, 3)]
                for pr, qc in order:
                    drain = emit_quarter(pr, qc, do_proj=(pr == 1),
                                         tail=((pr, qc) == order[-1]))
                # drain all remaining close-chain work, stage-batched so
                # each engine gets runs of independent work
                while chain2:
                    stage2(*chain2.popleft())
                while chain3:
                    stage3(*chain3.popleft())
                emit_filler(1 << 30)

    return nc


def _fix_matmul_waits(nc):
    """The TRN2 ISA events struct holds exactly ONE sync-wait per
    instruction and walrus codegen refuses instructions carrying more
    ("Too many sync wait commands").  Tile emits multi-wait instructions,
    so legalize: hoist excess waits onto single-wait NoOps inserted right
    before the instruction on the same engine -- engine FIFO order
    preserves the synchronization semantics."""
    import bass_rust
    import concourse.mybir as mybir

    n = 0
    for bb in nc.main_func.blocks:
        insts = bb.instructions
        i = 0
        while i < len(insts):
            ins = insts[i]
            si = getattr(ins, "sync_info", None)
            if si is not None and len(si.on_wait) >= 2:
                for w in si.on_wait[:-1]:
                    nop = mybir.InstNoOp(name=f"I-xwait-{n}", ins=[], outs=[])
                    nop.engine = ins.engine
                    nop.sync_info = bass_rust.SyncInfo(
                        on_wait=[w], on_update=[])
                    insts.insert(i, nop)
                    n += 1
                    i += 1
                ins.sync_info = bass_rust.SyncInfo(
                    on_wait=[si.on_wait[-1]], on_update=si.on_update)
            i += 1
    return n


def get_nc(legalize=True):
    key = ("nc", legalize)
    if key not in _CACHE:
        nc = _build_bass()
        if legalize:
            _fix_matmul_waits(nc)
        _CACHE[key] = nc
    return _CACHE[key]


def make_in_maps(x, W_q, W_k, W_v, W_proj):
    import ml_dtypes

    bf = ml_dtypes.bfloat16
    x = np.asarray(x, np.float32)
    W_q = np.asarray(W_q, np.float32)
    W_k = np.asarray(W_k, np.float32)
    W_v = np.asarray(W_v, np.float32)
    W_proj = np.asarray(W_proj, np.float32)

    mask = np.triu(np.full((P, P), -MASK_C, np.float32), k=1).astype(bf)
    ident = np.eye(P, dtype=bf)

    xTs = [np.ascontiguousarray(x[b].T).astype(bf) for b in range(2)]
    in_maps = []
    for core in range(N_CORES):
        b = core // 4
        g = core % 4
        rs = slice(g * 256, (g + 1) * 256)
        in_maps.append({
            "xT": xTs[b],
            "wq_t": np.ascontiguousarray(W_q[rs].T / 8.0).astype(bf),
            "wk_t": np.ascontiguousarray(W_k[rs].T).astype(bf),
            "wv_t": np.ascontiguousarray(W_v[rs].T).astype(bf),
            "wp_t": np.ascontiguousarray(W_proj[:, rs].T).astype(bf),
            "mask_lhsT": mask,
            "ident": ident,
        })
    return in_maps


def kernel(x, W_q, W_k, W_v, W_proj, _results_hook=None):
    from concourse.bass_utils import run_bass_kernel_spmd

    nc = get_nc()
    in_maps = make_in_maps(x, W_q, W_k, W_v, W_proj)
    res = run_bass_kernel_spmd(nc, in_maps, core_ids=list(range(N_CORES)))
    if _results_hook is not None:
        _results_hook(res)
    out = np.zeros((2, S, D), np.float32)
    for core in range(N_CORES):
        out[core // 4] += res.results[core]["out"].astype(np.float32)
    return out


if __name__ == "__main__":
    nc = get_nc()
    print("built ok; instructions:",
          sum(len(bb.instructions) for bb in nc.main_func.blocks))


# revision 71
# speedup vs baseline: 1.0286x; 1.0024x over previous
"""Causal multi-head attention (B=2, S=2048, D=1024, H=16) on 8 trn2
NeuronCores.

Sharding (head-parallel): core c handles batch c//4 and heads
4*(c%4) .. 4*(c%4)+3 (a 256-wide slice of the q/k/v feature dim).  W_proj is
tensor-parallel split along the head dim; each core emits a full-shape [S, D]
partial projection output (bf16); the host sums the 4 partials per batch.

Everything on-chip is bf16 (inputs converted on host): bf16 matmuls run at
1 cycle/row at any tile size, DMA bytes halve, and DVE 16-bit fast modes
apply.  Per-core structure:

  - x fed transposed ([d, s]) so the contraction lands on partitions;
    weights load as single DMAs (the SP sequencer serializes DMA issues)
  - q/k for head-pair 0 computed chunk-major so the PE tracks the x DMA
    chunk arrivals; pair-1 q/k and the v projection form a milestone-
    guarded "filler" stream paced evenly across the attention schedule
  - quarters run pair-interleaved ((0,0),(0,1),(1,0),(0,2),(1,1),(0,3),
    (1,2),(1,3)) so the serial ACT exp stream and the per-tile projection
    chains spread across the whole timeline
  - scoresT strips [sk, 2*sq] per head pair with causal raggedness;
    diagonal blocks masked by one bf16 matmul (strict-upper -1000 against
    identity); softmax needs no max subtraction (scores ~ N(0,1))
  - exp on ACT writes et (bf16); the AV matmul consumes et as lhsT
    (contraction = sk) so its output is attn[sq, dh] at only 65 rows per
    accumulation step (64 v cols + 1 ones column for the denominator).
    PSUM start=True lazily zeroes a whole 2KB bank, so only the first
    write into each bank per quarter carries it
  - per-tile normalization (reciprocal + per-partition-scalar muls), PE
    transpose back to [dh, sq], projection + output DMA chained per-tile;
    close-chains are emitted with a lag so the PE FIFO never reaches an
    instruction whose DVE/ACT input is fresh, and the final quarter's
    chains flip to the by-then-idle ACT engine
  - one 8-bank PSUM pool for the whole kernel; window-0 accumulators
    alias the attention-phase tags so there is no pool-transition barrier

The TRN2 ISA holds one sync-wait per instruction; Tile emits more, so
excess waits are hoisted onto same-engine NoOps after scheduling.
"""

import itertools
import os
from collections import deque

import numpy as np

# cache compiled executables (incl. the wrapped NEFF) across processes
os.environ.setdefault("JAX_COMPILATION_CACHE_DIR", "/tmp/jax_comp_cache")
os.environ.setdefault("JAX_PERSISTENT_CACHE_MIN_ENTRY_SIZE_BYTES", "0")
os.environ.setdefault("JAX_PERSISTENT_CACHE_MIN_COMPILE_TIME_SECS", "0")

S = 2048
D = 1024
P = 128
NT = S // P   # 16 sequence tiles
DC = D // P   # 8 contraction chunks
MASK_C = 1000.0
N_CORES = 8
AV_LAG = 3    # steps between emitting scores(j) and AV(j)

_CACHE = {}


def _build_bass():
    import concourse.bass as bass
    import concourse.tile as tile
    from concourse import mybir

    f32 = mybir.dt.float32
    bf16 = mybir.dt.bfloat16
    EXP = mybir.ActivationFunctionType.Exp

    nc = bass.Bass("TRN2")

    xT_d = nc.dram_tensor("xT", [D, S], bf16, kind="ExternalInput")
    wq_d = nc.dram_tensor("wq_t", [D, 256], bf16, kind="ExternalInput")
    wk_d = nc.dram_tensor("wk_t", [D, 256], bf16, kind="ExternalInput")
    wv_d = nc.dram_tensor("wv_t", [D, 256], bf16, kind="ExternalInput")
    wp_d = nc.dram_tensor("wp_t", [256, D], bf16, kind="ExternalInput")
    mask_d = nc.dram_tensor("mask_lhsT", [P, P], bf16, kind="ExternalInput")
    ident_d = nc.dram_tensor("ident", [P, P], bf16, kind="ExternalInput")
    out_d = nc.dram_tensor("out", [S, D], bf16, kind="ExternalOutput")

    with tile.TileContext(nc) as tc:
        with tc.tile_pool(name="persist", bufs=1) as persist:
            xTt = [persist.tile([P, S], bf16, name=f"xTt{c}", tag=f"xTt{c}")
                   for c in range(DC)]
            # weights as single tiles so each loads with ONE dma (the SP
            # sequencer serializes dma issues at 565ns each)
            wq_sb = persist.tile([P, DC, 256], bf16, name="wq_sb", tag="wq_sb")
            wk_sb = persist.tile([P, DC, 256], bf16, name="wk_sb", tag="wk_sb")
            wv_sb = persist.tile([P, DC, 256], bf16, name="wv_sb", tag="wv_sb")
            wp_sb = persist.tile([P, 2, D], bf16, name="wp_sb", tag="wp_sb")
            qT = [persist.tile([P, S], bf16, name=f"qT{p}", tag=f"qT{p}")
                  for p in range(2)]
            kT = [persist.tile([P, S], bf16, name=f"kT{p}", tag=f"kT{p}")
                  for p in range(2)]
            # per sk-tile: 4 heads x [64 v-cols | 1 ones-col]; the ones col
            # makes the AV matmul emit the softmax denominator at col 64
            v4e = [persist.tile([P, 4, 65], bf16, name=f"v4e{t}", tag=f"v4e{t}")
                   for t in range(NT)]
            # normalized attention, [sq, 2 heads x 64] per (pair, sq-tile)
            attn_sb = [[persist.tile([P, P], bf16, name=f"at{p}_{t}",
                                     tag=f"at{p}_{t}") for t in range(NT)]
                       for p in range(2)]
            attnT = [persist.tile([P, S], bf16, name=f"attnT{p}",
                                  tag=f"attnT{p}") for p in range(2)]
            mask_sb = persist.tile([P, P], bf16, name="mask_sb", tag="mask_sb")
            ident_sb = persist.tile([P, P], bf16, name="ident_sb",
                                    tag="ident_sb")

            # DMA order = consumption order; x chunked to pace the
            # chunk-major qk0 loop, weights combined into single transfers.
            wq_r = wq_d.rearrange("(c p) n -> p c n", p=P)
            wk_r = wk_d.rearrange("(c p) n -> p c n", p=P)
            wv_r = wv_d.rearrange("(c p) n -> p c n", p=P)
            wp_r = wp_d.rearrange("(c p) n -> p c n", p=P)
            nc.sync.dma_start(out=wq_sb[:, 0:4, :], in_=wq_r[:, 0:4, :])
            nc.sync.dma_start(out=xTt[0][:, 0:1024], in_=xT_d[0:P, 0:1024])
            nc.sync.dma_start(out=wq_sb[:, 4:8, :], in_=wq_r[:, 4:8, :])
            nc.sync.dma_start(out=xTt[0][:, 1024:S], in_=xT_d[0:P, 1024:S])
            nc.sync.dma_start(out=wk_sb[:], in_=wk_r)
            for c in range(1, DC):
                nc.sync.dma_start(out=xTt[c][:], in_=xT_d[c * P:(c + 1) * P, :])
            nc.sync.dma_start(out=mask_sb[:], in_=mask_d[:])
            nc.sync.dma_start(out=ident_sb[:], in_=ident_d[:])
            nc.sync.dma_start(out=wv_sb[:], in_=wv_r)
            nc.sync.dma_start(out=wp_sb[:], in_=wp_r)

            # One psum pool for the whole kernel (8 banks exactly):
            #   strip: 2 x [P,2,512] f32 (4 banks)  pa: 1 x [P,2,512] (2)
            #   aux1:  1 bank   aux2: 1 bank
            # Window 0 (pair-0 q/k, chunk-major) aliases its 8 psum
            # accumulators onto these same tags so there is no pool
            # transition barrier: the first window-A allocations just WAR on
            # the matching window-0 copies.
            with tc.tile_pool(name="asb", bufs=1) as asb, \
                 tc.tile_pool(name="ps", bufs=1, space="PSUM") as ps:

                qp01 = ps.tile([P, 2, 512], f32, name="qp01", tag="strip",
                               bufs=2)
                qp23 = ps.tile([P, 2, 512], f32, name="qp23", tag="strip",
                               bufs=2)
                kp23 = ps.tile([P, 2, 512], f32, name="kp23", tag="pa")
                kp0 = ps.tile([P, 512], f32, name="kp0", tag="aux1")
                kp1 = ps.tile([P, 512], f32, name="kp1", tag="aux2")
                qp = [qp01[:, 0, :], qp01[:, 1, :], qp23[:, 0, :],
                      qp23[:, 1, :]]
                kp = [kp0[:], kp1[:], kp23[:, 0, :], kp23[:, 1, :]]
                for c in range(DC - 1):
                    for n in range(4):
                        nc.tensor.matmul(
                            qp[n], lhsT=wq_sb[:, c, 0:P],
                            rhs=xTt[c][:, n * 512:(n + 1) * 512],
                            start=(c == 0), stop=False,
                            skip_group_check=True)
                    for n in range(4):
                        nc.tensor.matmul(
                            kp[n], lhsT=wk_sb[:, c, 0:P],
                            rhs=xTt[c][:, n * 512:(n + 1) * 512],
                            start=(c == 0), stop=False,
                            skip_group_check=True)
                # last chunk: finish k/q per n and copy out immediately
                # (k on DVE, q on ACT) so the copies overlap the next n's
                # matmuls and the psum slots free up front-to-back
                c = DC - 1
                for n in range(4):
                    nc.tensor.matmul(
                        kp[n], lhsT=wk_sb[:, c, 0:P],
                        rhs=xTt[c][:, n * 512:(n + 1) * 512],
                        start=False, stop=True, skip_group_check=True)
                    nc.tensor.matmul(
                        qp[n], lhsT=wq_sb[:, c, 0:P],
                        rhs=xTt[c][:, n * 512:(n + 1) * 512],
                        start=False, stop=True, skip_group_check=True)
                    nc.scalar.copy(kT[0][:, n * 512:(n + 1) * 512], kp[n])
                    nc.vector.tensor_copy(qT[0][:, n * 512:(n + 1) * 512],
                                          qp[n])

                def v_tiles(t0, t1):
                    for t in range(t0, t1):
                        vp = ps.tile([P, 256], f32, name="vp", tag="aux1")
                        for c in range(DC):
                            nc.tensor.matmul(
                                vp[:], lhsT=xTt[c][:, t * P:(t + 1) * P],
                                rhs=wv_sb[:, c, :],
                                start=(c == 0), stop=(c == DC - 1),
                                skip_group_check=True)
                            if c < DC - 1:
                                yield 256, None
                        nc.vector.tensor_copy(
                            v4e[t][:, :, 0:64],
                            vp.rearrange("p (h d) -> p h d", h=4))
                        nc.vector.memset(v4e[t][:, :, 64:65], 1.0)
                        yield 256, f"v{t}"

                def qk1_block(key, n):
                    dst, wsb = (kT, wk_sb) if key == "k" else (qT, wq_sb)
                    pp = ps.tile([P, 512], f32, name="pp", tag="aux2")
                    for c in range(DC):
                        nc.tensor.matmul(
                            pp[:], lhsT=wsb[:, c, P:2 * P],
                            rhs=xTt[c][:, n * 512:(n + 1) * 512],
                            start=(c == 0), stop=(c == DC - 1),
                            skip_group_check=True)
                        if c < DC - 1:
                            yield 512, None
                    nc.vector.tensor_copy(
                        dst[1][:, n * 512:(n + 1) * 512], pp[:])
                    yield 512, f"{key}1n{n}"

                def filler_stream():
                    # ordered to match the quarter schedule's consumption;
                    # need() force-drains on any shortfall
                    for g in range(4):
                        yield from v_tiles(4 * g, 4 * g + 4)
                        yield from qk1_block("k", g)
                        yield from qk1_block("q", g)

                filler = filler_stream()
                filler_done = [False]
                fill_ms = set()
                fill_left = [2 * 32768]   # total filler rows (v + qk1)
                steps_left = [104]        # in-loop + trailing steps

                def emit_filler(rows_target):
                    got = 0
                    while got < rows_target and not filler_done[0]:
                        try:
                            r, m = next(filler)
                            got += r
                            if m:
                                fill_ms.add(m)
                        except StopIteration:
                            filler_done[0] = True
                    fill_left[0] -= got
                    return got

                def emit_filler_paced():
                    # spread the remaining filler evenly over the remaining
                    # steps so late quarters keep PE-busy work too
                    steps_left[0] = max(steps_left[0] - 1, 1)
                    emit_filler(fill_left[0] // steps_left[0])

                def need(m):
                    # force-drain filler until milestone m has been emitted
                    # (emission-order dependency: the dependent instruction
                    # must come AFTER the work it reads, or it reads garbage)
                    while m not in fill_ms and not filler_done[0]:
                        emit_filler(1)

                # close-chain stages, deferred so PE never waits on a
                # just-issued DVE/ACT result: stage1 = norm (per-quarter,
                # reads pa), stage2 = transpose + attnT copy, stage3 =
                # proj + out.  chain2/3 carry only tile indices, so they
                # survive across quarters.
                chain2 = deque()
                chain3 = deque()

                def stage2(t, tail):
                    for p in range(2):
                        tp = ps.tile([P, P], bf16, name="tp", tag="aux1")
                        nc.tensor.transpose(
                            tp[:], attn_sb[p][t][:], ident_sb[:])
                        if tail and p == 0:
                            nc.scalar.copy(
                                attnT[p][:, t * P:(t + 1) * P], tp[:])
                        else:
                            nc.vector.tensor_copy(
                                attnT[p][:, t * P:(t + 1) * P], tp[:])
                    chain3.append((t, 0, tail))
                    chain3.append((t, 1, tail))

                def stage3(t, oc, tail):
                    pj = ps.tile([P, 512], f32, name="pj", tag="aux2")
                    for p in range(2):
                        nc.tensor.matmul(
                            pj[:],
                            lhsT=attnT[p][:, t * P:(t + 1) * P],
                            rhs=wp_sb[:, p, oc * 512:(oc + 1) * 512],
                            start=(p == 0), stop=(p == 1),
                            skip_group_check=True)
                    ob = asb.tile([P, 512], bf16, name="ob", tag="ob",
                                  bufs=8)
                    if oc == 1 and tail:
                        nc.scalar.copy(ob[:], pj[:])
                    else:
                        nc.vector.tensor_copy(ob[:], pj[:])
                    nc.sync.dma_start(
                        out=out_d[t * P:(t + 1) * P,
                                  oc * 512:(oc + 1) * 512],
                        in_=ob[:])

                def emit_quarter(pr, qc, do_proj, lag=AV_LAG, tail=False):
                    c0 = qc * 512
                    jmax = min(4 * qc + 3, NT - 1)
                    pa = ps.tile([P, 2, 512], f32, name="pa", tag="pa")
                    ets = {}
                    pend = deque()
                    chain1 = deque()

                    def stage1(t, tt):
                        rc = asb.tile([P, 2, 1], f32, name="rc", tag="rc",
                                      bufs=8)
                        nc.vector.reciprocal(
                            rc[:], pa[:, :, tt * 65 + 64:tt * 65 + 65])
                        # one fused multiply for both heads via a
                        # zero-stride broadcast of the reciprocals
                        nc.vector.tensor_mul(
                            attn_sb[pr][t].rearrange("p (h d) -> p h d", h=2),
                            pa[:, :, :].rearrange(
                                "p h w -> p h w")[:, :, tt * 65:tt * 65 + 64],
                            rc[:].broadcast_to([P, 2, 64]))
                        if do_proj:
                            chain2.append((t, tail))

                    def run_chains():
                        if chain3:
                            stage3(*chain3.popleft())
                        if chain2:
                            stage2(*chain2.popleft())
                        while chain1:
                            stage1(*chain1.popleft())

                    def emit_av(j):
                        need(f"v{j}")
                        et = ets.pop(j)
                        for t in range(max(j, 4 * qc), 4 * qc + 4):
                            tt = t - 4 * qc
                            for h in range(2):
                                # start=True lazily zeroes the WHOLE psum
                                # bank (2KB zero region), so only the first
                                # write into each h-bank per quarter may
                                # carry it; later windows zero-fill on
                                # first touch.
                                nc.tensor.matmul(
                                    pa[:, h, tt * 65:tt * 65 + 65],
                                    lhsT=et[:, h, t * P - c0:t * P - c0 + P],
                                    rhs=v4e[j][:, 2 * pr + h, :],
                                    start=(j == 0 and tt == 0),
                                    stop=(j == t),
                                    skip_group_check=True)
                            if j == t:
                                chain1.append((t, tt))

                    for j in range(jmax + 1):
                        if pr == 1:
                            need(f"k1n{j // 4}")
                            need(f"q1n{qc}")
                        w0 = j * P
                        lo = max(w0, c0)
                        w = c0 + 512 - lo
                        strip = ps.tile([P, 2, 512], f32, name="strip",
                                        tag="strip", bufs=2)
                        diag = j // 4 == qc
                        for h in range(2):
                            nc.tensor.matmul(
                                strip[:, h, lo - c0:lo - c0 + w],
                                lhsT=kT[pr][h * 64:(h + 1) * 64, w0:w0 + P],
                                rhs=qT[pr][h * 64:(h + 1) * 64, lo:lo + w],
                                start=True, stop=not diag,
                                skip_group_check=True)
                        if diag:
                            for h in range(2):
                                nc.tensor.matmul(
                                    strip[:, h, w0 - c0:w0 - c0 + P],
                                    lhsT=mask_sb[:], rhs=ident_sb[:],
                                    start=False, stop=True,
                                    skip_group_check=True)
                        et = asb.tile([P, 2, 512], bf16, name="et", tag="et",
                                      bufs=8)
                        nc.scalar.activation(
                            out=et[:, :, lo - c0:lo - c0 + w],
                            in_=strip[:, :, lo - c0:lo - c0 + w],
                            func=EXP)
                        ets[j] = et
                        pend.append(j)
                        if len(pend) > lag:
                            emit_av(pend.popleft())
                        run_chains()
                        emit_filler_paced()
                    while pend:
                        emit_av(pend.popleft())
                        run_chains()
                        emit_filler_paced()
                    return run_chains

                # pair-interleaved quarter order: spreads pair-1 exp (ACT)
                # and the per-tile projection chains across the whole
                # timeline instead of back-loading them
                drain = None
                order = [(0, 0), (0, 1), (1, 0), (0, 2), (1, 1), (0, 3),
                         (1, 2), (1, 3)]
                for pr, qc in order:
                    drain = emit_quarter(pr, qc, do_proj=(pr == 1),
                                         tail=((pr, qc) == order[-1]))
                # drain all remaining close-chain work, stage-batched so
                # each engine gets runs of independent work
                while chain2:
                    stage2(*chain2.popleft())
                while chain3:
                    stage3(*chain3.popleft())
                emit_filler(1 << 30)

    return nc


def _fix_matmul_waits(nc):
    """The TRN2 ISA events struct holds exactly ONE sync-wait per
    instruction and walrus codegen refuses instructions carrying more
    ("Too many sync wait commands").  Tile emits multi-wait instructions,
    so legalize: hoist excess waits onto single-wait NoOps inserted right
    before the instruction on the same engine -- engine FIFO order
    preserves the synchronization semantics."""
    import bass_rust
    import concourse.mybir as mybir

    n = 0
    for bb in nc.main_func.blocks:
        insts = bb.instructions
        i = 0
        while i < len(insts):
            ins = insts[i]
            si = getattr(ins, "sync_info", None)
            if si is not None and len(si.on_wait) >= 2:
                for w in si.on_wait[:-1]:
                    nop = mybir.InstNoOp(name=f"I-xwait-{n}", ins=[], outs=[])
                    nop.engine = ins.engine
                    nop.sync_info = bass_rust.SyncInfo(
                        on_wait=[w], on_update=[])
                    insts.insert(i, nop)
                    n += 1
                    i += 1
                ins.sync_info = bass_rust.SyncInfo(
                    on_wait=[si.on_wait[-1]], on_update=si.on_update)
            i += 1
    return n


def get_nc(legalize=True):
    key = ("nc", legalize)
    if key not in _CACHE:
        nc = _build_bass()
        if legalize:
            _fix_matmul_waits(nc)
        _CACHE[key] = nc
    return _CACHE[key]


def make_in_maps(x, W_q, W_k, W_v, W_proj):
    import ml_dtypes

    bf = ml_dtypes.bfloat16
    x = np.asarray(x, np.float32)
    W_q = np.asarray(W_q, np.float32)
    W_k = np.asarray(W_k, np.float32)
    W_v = np.asarray(W_v, np.float32)
    W_proj = np.asarray(W_proj, np.float32)

    mask = np.triu(np.full((P, P), -MASK_C, np.float32), k=1).astype(bf)
    ident = np.eye(P, dtype=bf)

    xTs = [np.ascontiguousarray(x[b].T).astype(bf) for b in range(2)]
    in_maps = []
    for core in range(N_CORES):
        b = core // 4
        g = core % 4
        rs = slice(g * 256, (g + 1) * 256)
        in_maps.append({
            "xT": xTs[b],
            "wq_t": np.ascontiguousarray(W_q[rs].T / 8.0).astype(bf),
            "wk_t": np.ascontiguousarray(W_k[rs].T).astype(bf),
            "wv_t": np.ascontiguousarray(W_v[rs].T).astype(bf),
            "wp_t": np.ascontiguousarray(W_proj[:, rs].T).astype(bf),
            "mask_lhsT": mask,
            "ident": ident,
        })
    return in_maps


def kernel(x, W_q, W_k, W_v, W_proj, _results_hook=None):
    from concourse.bass_utils import run_bass_kernel_spmd

    nc = get_nc()
    in_maps = make_in_maps(x, W_q, W_k, W_v, W_proj)
    res = run_bass_kernel_spmd(nc, in_maps, core_ids=list(range(N_CORES)))
    if _results_hook is not None:
        _results_hook(res)
    out = np.zeros((2, S, D), np.float32)
    for core in range(N_CORES):
        out[core // 4] += res.results[core]["out"].astype(np.float32)
    return out


if __name__ == "__main__":
    nc = get_nc()
    print("built ok; instructions:",
          sum(len(bb.instructions) for bb in nc.main_func.blocks))
